# revision 1
# baseline (speedup 1.0000x reference)
"""DeformConv3D Trainium2 Bass kernel (raw-bass, 8-core SPMD).

Algorithm per core (shard = one batch x 16 z-planes = 65536 voxels):
  1. offset conv: PE matmuls (fp32, M=32-padded, 4-way col-tiled PSUM)
  2. PE-transpose offsets into [128, NJ*3] field tiles
  3. DVE coordinate math -> per-voxel block index (8-parity 2x2x2-blocked
     bf16 copy of x, built on host) + 8 trilinear corner weights
  4. indirect-DMA gather: one 1KB block per voxel (all 8 corners), 128
     voxels per instruction -> G[p, j, dz, dy, dx, c]
  5. combine: S = U (*) G (DVE/GPSIMD), fold dz, fold dy (DVE adds);
     dx is folded by stacking W twice on the matmul contraction dim
  6. PE-transpose S to chan-major, matmul with Wstack -> +bias -> out

All synchronization is explicit (this toolchain encodes at most one
sem-wait per instruction, so Tile-generated code does not compile).
"""

import sys

import numpy as np
import ml_dtypes

import concourse.bass as bass
import concourse.mybir as mybir
from concourse.bass import AP, IndirectOffsetOnAxis
from concourse.bass_utils import run_bass_kernel_spmd

bf16 = ml_dtypes.bfloat16
f32 = mybir.dt.float32
bft = mybir.dt.bfloat16
i32 = mybir.dt.int32
Alu = mybir.AluOpType
Act = mybir.ActivationFunctionType

B, CIN, COUT, D, H, W = 2, 64, 64 * 2, 64, 64, 64
NCORE = 8
SH = D // (NCORE // B)      # 16 z-planes per core
NV = SH * H * W             # 65536 voxels per core
NJ = NV // 128              # 512 j-columns; voxel v = j*128 + p
NBLK = 8 * B * 32 * 32 * 32  # 524288 parity blocks
PADBLK = 64
K_CH = 8                    # j-columns (gather instructions) per chunk
NCHUNK = NJ // K_CH         # 64
CHV = K_CH * 128            # 1024 voxels per chunk

MUL_ON_POOL_EVERY = 1 << 30  # chunk % N == N-1 -> gpsimd does the big multiply

_PROGRAM = None
_RUNNER = None


def _build_program(repeat=1):
    nc = bass.Bass()

    xq_d = nc.declare_dram_parameter("xq", [NBLK + PADBLK, 512], bft, isOutput=False)
    xns_d = nc.declare_dram_parameter("xns", [CIN, NV], f32, isOutput=False)
    btile_d = nc.declare_dram_parameter("btile", [128, NJ * 3], f32, isOutput=False)
    rowb_d = nc.declare_dram_parameter("rowbase", [128, 1], f32, isOutput=False)
    wofft_d = nc.declare_dram_parameter("wofft", [64, 32], f32, isOutput=False)
    wstk_d = nc.declare_dram_parameter("wstack", [128, 128], bft, isOutput=False)
    bconv_d = nc.declare_dram_parameter("bconv", [128, 1], f32, isOutput=False)
    ident_d = nc.declare_dram_parameter("ident", [128, 128], f32, isOutput=False)
    identb_d = nc.declare_dram_parameter("identb", [128, 128], bft, isOutput=False)
    out_d = nc.declare_dram_parameter("out", [COUT, NV], bft, isOutput=True)

    ctxs = []

    def sb(name, shape, dtype):
        cm = nc.sbuf_tensor(name, shape, dtype)
        t = cm.__enter__()
        ctxs.append(cm)
        return t

    def ps(name, shape, dtype):
        cm = nc.psum_tensor(name, shape, dtype)
        t = cm.__enter__()
        ctxs.append(cm)
        return t

    def sem(name):
        cm = nc.semaphore(name)
        s = cm.__enter__()
        ctxs.append(cm)
        return s

    # constants
    btile = sb("sb_btile", [128, NJ * 3], f32)
    rowb = sb("sb_rowb", [128, 1], f32)
    wofft = sb("sb_wofft", [64, 32], f32)
    wstk = sb("sb_wstk", [128, 128], bft)
    bconv = sb("sb_bconv", [128, 1], f32)
    ident = sb("sb_ident", [128, 128], f32)
    identb = sb("sb_identb", [128, 128], bft)
    # phase A
    xcm = [sb(f"sb_xcm{i}", [64, 512], f32) for i in range(3)]
    stage = [sb(f"sb_stage{i}", [128, 512], f32) for i in range(2)]
    F = sb("sb_F", [128, NJ * 3], f32)
    # fields
    P = sb("sb_P", [128, NJ * 3], f32)
    Fr = sb("sb_Fr", [128, NJ * 3], f32)
    tA = sb("sb_tA", [128, NJ], f32)
    tB = sb("sb_tB", [128, NJ], f32)
    tC = sb("sb_tC", [128, NJ], f32)
    tD = sb("sb_tD", [128, NJ], f32)
    wz0 = sb("sb_wz0", [128, NJ], f32)
    wy0 = sb("sb_wy0", [128, NJ], f32)
    w4 = {zy: sb(f"sb_w4_{zy[0]}{zy[1]}", [128, NJ], f32)
          for zy in [(0, 0), (0, 1), (1, 0), (1, 1)]}
    I = sb("sb_I", [128, NJ], i32)
    Ibig = sb("sb_Ibig", [128, NJ * 3], i32)
    tE3 = sb("sb_tE3", [128, NJ * 3], f32)
    U = sb("sb_U", [128, 8 * NJ], bft)
    # main loop
    G = [sb(f"sb_G{i}", [128, K_CH * 512], bft) for i in range(4)]
    R1 = [sb(f"sb_R1_{i}", [128, K_CH * 256], bft) for i in range(2)]
    R2 = [sb(f"sb_R2_{i}", [128, K_CH * 128], bft) for i in range(2)]
    scm = [sb(f"sb_scm{i}", [128, CHV], bft) for i in range(2)]
    ost = [sb(f"sb_ost{i}", [128, CHV], bft) for i in range(4)]

    pofs = [ps(f"sb_pofs{i}", [128, 512], f32) for i in range(2)]
    ptr = [ps(f"sb_ptr{i}", [128, 128], f32) for i in range(2)]
    pT = [ps(f"sb_pT{i}", [128, 512], bft) for i in range(2)]
    pO = [ps(f"sb_pO{i}", [128, 512], f32) for i in range(2)]

    s_ld = sem("s_ld")
    s_xcm = sem("s_xcm")
    s_offm = sem("s_offm")
    s_offp = sem("s_offp")
    s_stg = sem("s_stg")
    s_trp = sem("s_trp")
    s_ext = sem("s_ext")
    s_fld = sem("s_fld")
    s_gth = sem("s_gth")
    s_mulp = sem("s_mulp")
    s_cmb = sem("s_cmb")
    s_trpS = sem("s_trpS")
    s_exS = sem("s_exS")
    s_mm = sem("s_mm")
    s_act = sem("s_act")
    s_out = sem("s_out")

    NT = NJ // 16  # 32 stage tiles in phase A

    def wge(eng, s, n):
        if n > 0:
            eng.wait_ge(s, n)

    pool_mul_chunks = [
        cc for cc in range(NCHUNK * repeat)
        if cc % MUL_ON_POOL_EVERY == MUL_ON_POOL_EVERY - 1
    ]

    def g_views(t):
        g5 = t[:].rearrange("p (j dz r) -> p j dz r", dz=2, r=256)
        return g5

    with nc.Block() as block:

        # ---------------- SP: all HWDGE DMA ----------------
        @block.sync
        def _(sync):
            for name, dst, src in [
                ("btile", btile, btile_d), ("rowb", rowb, rowb_d),
                ("wofft", wofft, wofft_d), ("wstk", wstk, wstk_d),
                ("bconv", bconv, bconv_d), ("ident", ident, ident_d),
                ("identb", identb, identb_d),
            ]:
                sync.dma_start(out=dst[:], in_=src[:]).then_inc(s_ld, 16)
            for i in range(4 * NT):
                wge(sync, s_offm, i - 2)
                sync.dma_start(
                    out=xcm[i % 3][:], in_=xns_d[:, 512 * i : 512 * i + 512]
                ).then_inc(s_xcm, 16)
            for cc in range(NCHUNK * repeat):
                ccd = cc % NCHUNK
                wge(sync, s_act, 2 * cc + 2)
                sync.dma_start(
                    out=out_d[:, CHV * ccd : CHV * ccd + CHV], in_=ost[cc % 4][:]
                ).then_inc(s_out, 16)

        # ---------------- PE ----------------
        @block.tensor
        def _(pe):
            wge(pe, s_ld, 112)
            for t in range(NT):
                for g in range(4):
                    i = 4 * t + g
                    wge(pe, s_xcm, 16 * (i + 1))
                    if g == 0:
                        wge(pe, s_stg, t - 1)
                    nc.tensor.matmul(
                        out=pofs[t % 2][32 * g : 32 * g + 32, :],
                        lhsT=wofft[:],
                        rhs=xcm[i % 3][:],
                        start=True,
                        stop=True,
                        tile_position=(0, 32 * g),
                    ).then_inc(s_offm, 1)
                # transposes of stage tile t-1
                if t >= 1:
                    tau = t - 1
                    for bb in range(4):
                        k = 4 * tau + bb
                        wge(pe, s_stg, tau + 1)
                        wge(pe, s_ext, k - 1)
                        nc.tensor.transpose(
                            out=ptr[k % 2][:],
                            in_=stage[tau % 2][:, 128 * bb : 128 * bb + 128],
                            identity=ident[:],
                        ).then_inc(s_trp, 1)
            tau = NT - 1
            for bb in range(4):
                k = 4 * tau + bb
                wge(pe, s_stg, tau + 1)
                wge(pe, s_ext, k - 1)
                nc.tensor.transpose(
                    out=ptr[k % 2][:],
                    in_=stage[tau % 2][:, 128 * bb : 128 * bb + 128],
                    identity=ident[:],
                ).then_inc(s_trp, 1)

            # phase C: S transposes + main conv
            for cc in range(NCHUNK * repeat):
                wge(pe, s_cmb, cc + 1)
                for q in range(8):
                    bank = q // 4
                    wge(pe, s_exS, 2 * cc + bank - 1)
                    nc.tensor.transpose(
                        out=pT[bank][:, 128 * (q % 4) : 128 * (q % 4) + 128],
                        in_=R2[cc % 2][:, 128 * q : 128 * q + 128],
                        identity=identb[:],
                    ).then_inc(s_trpS, 1)
                for m in range(2):
                    k = 2 * cc + m
                    wge(pe, s_exS, k + 1)
                    wge(pe, s_act, k - 1)
                    nc.tensor.matmul(
                        out=pO[k % 2][:],
                        lhsT=wstk[:],
                        rhs=scm[cc % 2][:, 512 * m : 512 * m + 512],
                        start=True,
                        stop=True,
                    ).then_inc(s_mm, 1)

        # ---------------- DVE ----------------
        @block.vector
        def _(dve):
            # fields
            wge(dve, s_ext, 4 * NT)  # 128 extracts
            wge(dve, s_ld, 112)
            v = nc.vector
            v.tensor_add(out=P[:], in0=F[:], in1=btile[:])
            v.tensor_scalar(out=P[:], in0=P[:], scalar1=0.0, scalar2=63.0,
                            op0=Alu.max, op1=Alu.min)
            # floor via i32 round-trip + is_gt fixup (robust to cast rounding)
            v.tensor_copy(out=Ibig[:], in_=P[:])
            v.tensor_copy(out=Fr[:], in_=Ibig[:])
            v.tensor_tensor(out=tE3[:], in0=Fr[:], in1=P[:], op=Alu.is_gt)
            v.tensor_sub(out=Fr[:], in0=Fr[:], in1=tE3[:])   # Fr = floor(P)
            v.tensor_sub(out=P[:], in0=P[:], in1=Fr[:])      # P = frac
            v.tensor_copy(out=tE3[:], in_=Fr[:])
            v.tensor_copy(out=Fr[:], in_=P[:])               # Fr = frac
            v.tensor_copy(out=P[:], in_=tE3[:])              # P = floor

            def comp(tile, c):
                return tile[:].rearrange("p (j c) -> p j c", c=3)[:, :, c]

            ix0, iy0, iz0 = comp(P, 0), comp(P, 1), comp(P, 2)
            fx, fy, fz = comp(Fr, 0), comp(Fr, 1), comp(Fr, 2)

            # parity bits and halved coords; accumulate block index in tA
            # tA = sel*65536 + Z*1024 + Y*32 + X + rowbase
            # Z = (iz0 - hz)/2 etc, sel = hz*4 + hy*2 + hx
            Ism = Ibig[:].rearrange("p (j c) -> p j c", c=3)[:, :, 0]
            tE1 = tE3[:].rearrange("p (j c) -> p j c", c=3)[:, :, 0]

            def halve(coord, Zf_out, h_out):
                # Zf = floor(coord/2); h = coord - 2*Zf   (exact small ints)
                v.tensor_scalar(out=tD[:], in0=coord, scalar1=0.5, scalar2=None,
                                op0=Alu.mult)
                v.tensor_copy(out=Ism, in_=tD[:])
                v.tensor_copy(out=Zf_out, in_=Ism)
                v.tensor_tensor(out=tE1, in0=Zf_out, in1=tD[:], op=Alu.is_gt)
                v.tensor_sub(out=Zf_out, in0=Zf_out, in1=tE1)
                v.tensor_scalar(out=h_out, in0=Zf_out, scalar1=-2.0,
                                scalar2=None, op0=Alu.mult)
                v.tensor_add(out=h_out, in0=h_out, in1=coord)

            # z: tA accumulates hz*262144 + Z*512*?  (block idx parts)
            halve(iz0, tC[:], tB[:])
            v.tensor_scalar(out=tA[:], in0=tB[:], scalar1=262144.0, scalar2=None,
                            op0=Alu.mult)
            v.tensor_scalar(out=tC[:], in0=tC[:], scalar1=1024.0, scalar2=None,
                            op0=Alu.mult)
            v.tensor_add(out=tA[:], in0=tA[:], in1=tC[:])
            halve(iy0, tC[:], tB[:])
            v.tensor_scalar(out=tB[:], in0=tB[:], scalar1=131072.0, scalar2=None,
                            op0=Alu.mult)
            v.tensor_add(out=tA[:], in0=tA[:], in1=tB[:])
            v.tensor_scalar(out=tC[:], in0=tC[:], scalar1=32.0, scalar2=None,
                            op0=Alu.mult)
            v.tensor_add(out=tA[:], in0=tA[:], in1=tC[:])
            halve(ix0, tC[:], tB[:])
            v.tensor_scalar(out=tB[:], in0=tB[:], scalar1=65536.0, scalar2=None,
                            op0=Alu.mult)
            v.tensor_add(out=tA[:], in0=tA[:], in1=tB[:])
            v.tensor_add(out=tA[:], in0=tA[:], in1=tC[:])
            v.tensor_scalar(out=tA[:], in0=tA[:], scalar1=rowb[:, 0:1],
                            scalar2=None, op0=Alu.add)
            v.tensor_copy(out=I[:], in_=tA[:])

            # weights
            v.tensor_scalar(out=wz0[:], in0=fz, scalar1=-1.0, scalar2=1.0,
                            op0=Alu.mult, op1=Alu.add)
            v.tensor_scalar(out=wy0[:], in0=fy, scalar1=-1.0, scalar2=1.0,
                            op0=Alu.mult, op1=Alu.add)
            v.tensor_mul(out=w4[(0, 0)][:], in0=wz0[:], in1=wy0[:])
            v.tensor_sub(out=w4[(0, 1)][:], in0=wz0[:], in1=w4[(0, 0)][:])
            v.tensor_sub(out=w4[(1, 0)][:], in0=wy0[:], in1=w4[(0, 0)][:])
            v.tensor_sub(out=w4[(1, 1)][:], in0=fz, in1=w4[(1, 0)][:])
            uv = U[:].rearrange("p (j s) -> p j s", s=8)
            last = None
            for (dz, dy), wt in w4.items():
                # u1 = w*fx -> slot dz*4+dy*2+1 ; u0 = w - u1 -> slot dz*4+dy*2
                v.tensor_mul(out=tB[:], in0=wt[:], in1=fx)
                v.tensor_sub(out=tC[:], in0=wt[:], in1=tB[:])
                v.tensor_copy(out=uv[:, :, 4 * dz + 2 * dy + 1], in_=tB[:])
                last = v.tensor_copy(out=uv[:, :, 4 * dz + 2 * dy], in_=tC[:])
            last.then_inc(s_fld, 1)

            # main loop: combine
            npool = 0
            for cc in range(NCHUNK * repeat):
                ccd = cc % NCHUNK
                gt = G[cc % 4]
                wge(dve, s_gth, 128 * (cc + 1))
                uslice = U[:, 8 * K_CH * ccd : 8 * K_CH * ccd + 8 * K_CH]
                ub = AP(uslice.tensor, uslice.offset, uslice.ap + [[0, 64]])
                gv = gt[:].rearrange("p (js c) -> p js c", c=64)
                if cc in pool_mul_chunks:
                    npool += 1
                    wge(dve, s_mulp, npool)
                else:
                    nc.vector.tensor_tensor(out=gv, in0=gv, in1=ub, op=Alu.mult)
                # R1/R2[cc%2] free once PE consumed chunk cc-2's transposes
                wge(dve, s_trpS, 8 * cc - 8)
                g5 = gt[:].rearrange("p (j dz r) -> p j dz r", dz=2, r=256)
                r1v = R1[cc % 2][:].rearrange("p (j dy r) -> p j dy r", dy=2, r=128)
                nc.vector.tensor_add(
                    out=R1[cc % 2][:].rearrange("p (j r) -> p j r", r=256),
                    in0=g5[:, :, 0, :], in1=g5[:, :, 1, :])
                nc.vector.tensor_add(
                    out=R2[cc % 2][:].rearrange("p (j r) -> p j r", r=128),
                    in0=r1v[:, :, 0, :], in1=r1v[:, :, 1, :]).then_inc(s_cmb, 1)

        # ---------------- ACT ----------------
        @block.scalar
        def _(act):
            def extract(k):
                wge(act, s_trp, k + 1)
                t, bb = k // 4, k % 4
                src = ptr[k % 2][:].rearrange("p (g r) -> p g r", r=32)[:, :, 0:3]
                col = 48 * t + 3 * bb
                fap = F[:]
                dst = AP(fap.tensor, fap.offset + col,
                         [fap.ap[0], [12, 4], [1, 3]])
                nc.scalar.copy(out=dst, in_=src).then_inc(s_ext, 1)

            for t in range(NT):
                wge(act, s_offm, 4 * t + 4)
                wge(act, s_trp, 4 * t - 4)
                nc.scalar.copy(
                    out=stage[t % 2][:], in_=pofs[t % 2][:]
                ).then_inc(s_stg, 1)
                if t >= 1:
                    for bb in range(4):
                        extract(4 * (t - 1) + bb)
            for bb in range(4):
                extract(4 * (NT - 1) + bb)

            for cc in range(NCHUNK * repeat):
                # S-transpose exits: pT -> scm (bf16)
                for bank in range(2):
                    wge(act, s_trpS, 8 * cc + 4 * (bank + 1))
                    wge(act, s_mm, 2 * cc - 2)
                    nc.scalar.copy(
                        out=scm[cc % 2][:, 512 * bank : 512 * bank + 512],
                        in_=pT[bank][:],
                    ).then_inc(s_exS, 1)
                # out exits
                for m in range(2):
                    k = 2 * cc + m
                    wge(act, s_mm, k + 1)
                    wge(act, s_out, 16 * (cc - 3))
                    nc.scalar.activation(
                        out=ost[cc % 4][:, 512 * m : 512 * m + 512],
                        in_=pO[k % 2][:],
                        func=Act.Identity,
                        bias=bconv[:, 0:1],
                        scale=1.0,
                    ).then_inc(s_act, 1)

        # ---------------- POOL ----------------
        @block.gpsimd
        def _(pool):
            wge(pool, s_fld, 1)
            npool = 0
            for cc in range(NCHUNK * repeat):
                ccd = cc % NCHUNK
                wge(pool, s_cmb, cc - 3)
                for jj in range(K_CH):
                    j = K_CH * ccd + jj
                    pool.indirect_dma_start(
                        out=G[cc % 4][:, 512 * jj : 512 * jj + 512],
                        out_offset=None,
                        in_=xq_d[:],
                        in_offset=IndirectOffsetOnAxis(ap=I[:, j : j + 1], axis=0),
                    ).then_inc(s_gth, 16)
                if cc in pool_mul_chunks:
                    npool += 1
                    wge(pool, s_gth, 128 * (cc + 1))
                    gt = G[cc % 4]
                    uslice = U[:, 8 * K_CH * ccd : 8 * K_CH * ccd + 8 * K_CH]
                    ub = AP(uslice.tensor, uslice.offset, uslice.ap + [[0, 64]])
                    gv = gt[:].rearrange("p (js c) -> p js c", c=64)
                    nc.gpsimd.tensor_tensor(
                        out=gv, in0=gv, in1=ub, op=Alu.mult
                    ).then_inc(s_mulp, 1)

    for cm in reversed(ctxs):
        cm.__exit__(None, None, None)
    return nc


def _get_program():
    global _PROGRAM
    if _PROGRAM is None:
        _PROGRAM = _build_program()
    return _PROGRAM


def build_bench(repeat):
    return _build_program(repeat=repeat)


def _prep_inputs(x, w_off, b_off, w_conv, b_conv):
    x = np.ascontiguousarray(np.asarray(x, np.float32))
    w_off = np.asarray(w_off, np.float32)
    b_off = np.asarray(b_off, np.float32)
    w_conv = np.asarray(w_conv, np.float32)
    b_conv = np.asarray(b_conv, np.float32)

    # 8-parity 2x2x2-blocked bf16 copies of x
    xb = x.transpose(0, 2, 3, 4, 1).astype(bf16)  # [B, D, H, W, C]
    xpad = np.zeros((B, D + 2, H + 2, W + 2, CIN), bf16)
    xpad[:, :D, :H, :W] = xb
    xq = np.zeros((NBLK + PADBLK, 512), bf16)
    blocks_per_sel = B * 32 * 32 * 32
    for sel in range(8):
        pz, py, px = (sel >> 2) & 1, (sel >> 1) & 1, sel & 1
        v = xpad[:, pz : pz + 64, py : py + 64, px : px + 64, :]
        v = v.reshape(B, 32, 2, 32, 2, 32, 2, CIN)
        v = v.transpose(0, 1, 3, 5, 2, 4, 6, 7)  # B,Z,Y,X,dz,dy,dx,C
        xq[sel * blocks_per_sel : (sel + 1) * blocks_per_sel] = v.reshape(
            blocks_per_sel, 512
        )

    wofft = np.zeros((64, 32), np.float32)
    wofft[:, :3] = (w_off * 32.0).T
    wstack = np.concatenate([w_conv.T, w_conv.T], axis=0).astype(bf16)
    bconv = np.ascontiguousarray(b_conv.reshape(COUT, 1))
    ident = np.eye(128, dtype=np.float32)
    identb = ident.astype(bf16)

    in_maps = []
    for core in range(NCORE):
        b = core // (NCORE // B)
        z0 = (core % (NCORE // B)) * SH
        xns = np.ascontiguousarray(x[b, :, z0 : z0 + SH].reshape(CIN, NV))
        v = np.arange(NV)
        zz = z0 + v // (H * W)
        yy = (v // W) % H
        xx = v % W
        base = np.stack(
            [
                64.0 * xx / 63.0 - 0.5 + 32.0 * b_off[0],
                64.0 * yy / 63.0 - 0.5 + 32.0 * b_off[1],
                64.0 * zz / 63.0 - 0.5 + 32.0 * b_off[2],
            ],
            axis=1,
        ).astype(np.float32)
        btile = np.ascontiguousarray(
            base.reshape(NJ, 128, 3).transpose(1, 0, 2).reshape(128, NJ * 3)
        )
        rowbase = np.full((128, 1), b * 32768.0, np.float32)
        in_maps.append(
            {
                "xq": xq,
                "xns": xns,
                "btile": btile,
                "rowbase": rowbase,
                "wofft": wofft,
                "wstack": wstack,
                "bconv": bconv,
                "ident": ident,
                "identb": identb,
            }
        )
    return in_maps


def _assemble(results):
    out = np.zeros((B, COUT, D, H, W), np.float32)
    for core in range(NCORE):
        b = core // (NCORE // B)
        z0 = (core % (NCORE // B)) * SH
        out[b, :, z0 : z0 + SH] = (
            results[core]["out"].astype(np.float32).reshape(COUT, SH, H, W)
        )
    return out


def kernel(x, w_off, b_off, w_conv, b_conv):
    nc = _get_program()
    in_maps = _prep_inputs(x, w_off, b_off, w_conv, b_conv)
    res = run_bass_kernel_spmd(nc, in_maps, list(range(NCORE)))
    return _assemble(res.results)

